# revision 19
# baseline (speedup 1.0000x reference)
"""Causal temporal attention (CausalGroupNorm + per-pixel temporal attention)
on 8 Trainium2 NeuronCores.

Sharding: data-parallel over the B*H*W pixel pseudo-batch. Core i handles
batch b = i//2 and h-rows [16*(i%2), 16*(i%2)+16) -- 512 pixels per core,
each with a [C=512, T=32] temporal sequence. The CxC projection weights are
replicated (pre-transposed / gamma-folded on host).

Per-core pipeline (single Tile kernel):
  Phase 1: GroupNorm stats. Stream x tiles, colsum x and x^2 on the PE
           (lhsT = ones), per-t partial sums -> [1, 64] = [sum | sumsq].
  Phase 2: pairwise AllReduce of the [1, 64] stats (cores sharing a batch),
           derive rstd[t], mean[t]*rstd[t]; broadcast across partitions via
           a K=1 PE matmul.
  Phase 3: stream 8 blocks of 64 pixels:
           h = x*r - m*r (pixel-major columns) -> q,k GEMMs, vT per group,
           S = q^T k (fp32r, paired groups for 256-wide rhs), masked softmax
           (ACT exp + fused rowsum), P transpose on PE, out = vT^T @ Pt,
           proj GEMM + residual add into the x tile, DMA out.
"""

import sys
import os

sys.path.insert(0, "/opt/trn_rl_repo")

import numpy as np

C = 512
T = 32
HL = 16          # h-rows per core
W = 32
NL = HL * W      # pixels per core = 512
PB = 64          # pixels per block
NB = NL // PB    # 8 blocks
CC = C // 128    # 4 chunks
NELEM = float(C * 2 * NL)  # elements per (b,t) frame for the group norm (C*H*W)
EPS = 1e-6

_CACHE = {}


def _build(collective=True):
    from concourse import bacc, tile, mybir, bass

    f32 = mybir.dt.float32
    f32r = mybir.dt.float32r
    bf16 = mybir.dt.bfloat16
    Alu = mybir.AluOpType
    Act = mybir.ActivationFunctionType

    nc = bacc.Bacc("TRN2", target_bir_lowering=False, debug=False, num_devices=8)

    x_d = nc.dram_tensor("x", [C, T, HL, W], f32, kind="ExternalInput").ap()
    wqt_d = nc.dram_tensor("wqt", [C, C], f32, kind="ExternalInput").ap()
    wkt_d = nc.dram_tensor("wkt", [C, C], f32, kind="ExternalInput").ap()
    wvt_d = nc.dram_tensor("wvt", [C, C], f32, kind="ExternalInput").ap()
    wpt_d = nc.dram_tensor("wpt", [C, C], f32, kind="ExternalInput").ap()
    bq_d = nc.dram_tensor("bq", [C], f32, kind="ExternalInput").ap()
    bk_d = nc.dram_tensor("bk", [C], f32, kind="ExternalInput").ap()
    bvb_d = nc.dram_tensor("bvb", [128, C], f32, kind="ExternalInput").ap()
    mask_d = nc.dram_tensor("mask", [128, 128], f32, kind="ExternalInput").ap()
    ident_d = nc.dram_tensor("ident", [128, 128], mybir.dt.bfloat16, kind="ExternalInput").ap()
    ones_d = nc.dram_tensor("ones", [128, 32], f32, kind="ExternalInput").ap()
    g1q_d = nc.dram_tensor("g1q", [1, C], f32, kind="ExternalInput").ap()
    g1k_d = nc.dram_tensor("g1k", [1, C], f32, kind="ExternalInput").ap()
    g1v_d = nc.dram_tensor("g1v", [1, C], f32, kind="ExternalInput").ap()
    y_d = nc.dram_tensor("y", [C, T, HL, W], f32, kind="ExternalOutput").ap()

    xv = x_d.rearrange("c t h w -> c t (h w)")   # [512, 32, 512]
    yv = y_d.rearrange("c t h w -> c t (h w)")

    def ap3(base, off, dims):
        return bass.AP(tensor=base.tensor, offset=base.offset + off, ap=[base.ap[0]] + dims)

    with tile.TileContext(nc) as tc:
        from contextlib import ExitStack

        with ExitStack() as ctx:
            persist = ctx.enter_context(tc.tile_pool(name="persist", bufs=1))

            # ---- constants / weights
            wq_t = persist.tile([128, CC, C], f32r, tag="wq")
            nc.sync.dma_start(out=wq_t, in_=wqt_d.rearrange("(cc p) o -> p cc o", p=128).bitcast(f32r))
            wk_t = persist.tile([128, CC, C], f32r, tag="wk")
            nc.sync.dma_start(out=wk_t, in_=wkt_d.rearrange("(cc p) o -> p cc o", p=128).bitcast(f32r))
            wv_t = persist.tile([128, CC, C], f32r, tag="wv")
            nc.sync.dma_start(out=wv_t, in_=wvt_d.rearrange("(cc p) o -> p cc o", p=128).bitcast(f32r))
            wp_t = persist.tile([128, CC, C], f32r, tag="wp")
            nc.sync.dma_start(out=wp_t, in_=wpt_d.rearrange("(cc p) o -> p cc o", p=128).bitcast(f32r))
            bq_t = persist.tile([128, CC], f32, tag="bq")
            nc.sync.dma_start(out=bq_t, in_=bq_d.rearrange("(cc p) -> p cc", p=128))
            bk_t = persist.tile([128, CC], f32, tag="bk")
            nc.sync.dma_start(out=bk_t, in_=bk_d.rearrange("(cc p) -> p cc", p=128))
            bvb_t = persist.tile([128, C], f32, tag="bvb")
            nc.sync.dma_start(out=bvb_t, in_=bvb_d)
            mask_t = persist.tile([128, 128], f32, tag="mask")
            nc.sync.dma_start(out=mask_t, in_=mask_d)
            ident_t = persist.tile([128, 128], bf16, tag="ident")
            nc.sync.dma_start(out=ident_t, in_=ident_d)
            g1q_t = persist.tile([1, C], f32, tag="g1q")
            nc.sync.dma_start(out=g1q_t, in_=g1q_d)
            g1k_t = persist.tile([1, C], f32, tag="g1k")
            nc.sync.dma_start(out=g1k_t, in_=g1k_d)
            g1v_t = persist.tile([1, C], f32, tag="g1v")
            nc.sync.dma_start(out=g1v_t, in_=g1v_d)

            ones_t = persist.tile([128, 32], f32r, tag="ones")
            nc.sync.dma_start(out=ones_t, in_=ones_d.bitcast(f32r))
            ones1_t = persist.tile([1, 128], f32, tag="ones1")
            nc.vector.memset(ones1_t, 1.0)
            eps_t = persist.tile([1, 1], f32, tag="eps")
            nc.vector.memset(eps_t, EPS)

            # stats accumulators [32(dummy), 32 t] each, zeroed
            acc1_t = persist.tile([32, T], f32, tag="acc1")
            nc.vector.memset(acc1_t, 0.0)
            acc2_t = persist.tile([32, T], f32, tag="acc2")
            nc.vector.memset(acc2_t, 0.0)

            # persist broadcast of rstd | mean*rstd  [128, 64]
            rmrb_t = persist.tile([128, 64], f32, tag="rmrb")
            # per-(o,t) corrections: q/k evac subtracts outer(g1, mr) - bias
            G1Q_t = persist.tile([128, CC, T], f32, tag="G1Q")
            G1K_t = persist.tile([128, CC, T], f32, tag="G1K")
            # vT-row correction: outer(mr_rowpattern, g1v) - bvb  [128, 512]
            CV_t = persist.tile([128, C], f32, tag="CV")

            # ================= Phase 1: stats =================
            with ExitStack() as p1:
                sb1 = p1.enter_context(tc.tile_pool(name="p1sb", bufs=1))
                ps1 = p1.enter_context(tc.tile_pool(name="p1ps", bufs=6, space="PSUM"))

                P1B = 128
                for blk in range(NL // P1B):
                    x1 = []
                    for cc in range(CC):
                        x1_t = sb1.tile([128, T, P1B], f32r, tag="x1", bufs=8)
                        nc.sync.dma_start(
                            out=x1_t,
                            in_=xv[cc * 128:(cc + 1) * 128, :, P1B * blk:P1B * (blk + 1)].bitcast(f32r),
                        )
                        x1.append(x1_t.rearrange("p t w -> p (t w)"))
                    for ns in range(T * P1B // 512):
                        sl = slice(512 * ns, 512 * (ns + 1))
                        cs_ps = ps1.tile([32, 512], f32, tag="cs")
                        sq_ps = ps1.tile([32, 512], f32, tag="cs")
                        for cc in range(CC):
                            nc.tensor.matmul(
                                cs_ps, lhsT=ones_t, rhs=x1[cc][:, sl],
                                start=(cc == 0), stop=(cc == CC - 1))
                        for cc in range(CC):
                            sq_t = sb1.tile([128, 512], f32r, tag="xsq", bufs=4)
                            nc.scalar.activation(out=sq_t, in_=x1[cc][:, sl],
                                                 func=Act.Square)
                            nc.tensor.matmul(
                                sq_ps, lhsT=ones_t, rhs=sq_t,
                                start=(cc == 0), stop=(cc == CC - 1))
                        for ps, acc in ((cs_ps, acc1_t), (sq_ps, acc2_t)):
                            red_t = sb1.tile([32, 4], f32, tag="red", bufs=4)
                            nc.vector.reduce_sum(
                                out=red_t,
                                in_=ps.rearrange("p (t w) -> p t w", t=4),
                                axis=mybir.AxisListType.X,
                            )
                            nc.vector.tensor_tensor(
                                out=acc[:, 4 * ns:4 * (ns + 1)],
                                in0=acc[:, 4 * ns:4 * (ns + 1)],
                                in1=red_t,
                                op=Alu.add,
                            )

            # ================= Phase 2: allreduce + derive =================
            with ExitStack() as p2:
                sb2 = p2.enter_context(tc.tile_pool(name="p2sb", bufs=1))
                ps2 = p2.enter_context(tc.tile_pool(name="p2ps", bufs=1, space="PSUM"))
                dram = p2.enter_context(tc.tile_pool(name="p2dram", bufs=1, space="DRAM"))

                stats_t = sb2.tile([1, 64], f32, tag="stats")
                nc.vector.tensor_copy(out=stats_t[:, 0:32], in_=acc1_t[0:1, :])
                nc.vector.tensor_copy(out=stats_t[:, 32:64], in_=acc2_t[0:1, :])

                st_in = dram.tile([1, 64], f32)
                st_out = dram.tile([1, 64], f32)
                nc.gpsimd.dma_start(out=st_in, in_=stats_t)
                if collective:
                    nc.gpsimd.collective_compute(
                        "AllReduce",
                        Alu.add,
                        replica_groups=[[0, 1], [2, 3], [4, 5], [6, 7]],
                        ins=[st_in.opt()],
                        outs=[st_out.opt()],
                    )
                else:
                    nc.gpsimd.dma_start(out=st_out, in_=st_in)
                vr_t = sb2.tile([1, 64], f32, tag="vr")
                nc.gpsimd.dma_start(out=vr_t, in_=st_out)

                # mean = S1/N ; e2 = S2/N ; var = e2 - mean^2
                # rm_t = [ rstd(32) | mean*rstd(32) ]
                mean_t = sb2.tile([1, 32], f32, tag="mean")
                nc.scalar.mul(out=mean_t, in_=vr_t[:, 0:32], mul=1.0 / NELEM)
                var_t = sb2.tile([1, 32], f32, tag="var")
                nc.scalar.mul(out=var_t, in_=vr_t[:, 32:64], mul=1.0 / NELEM)
                msq_t = sb2.tile([1, 32], f32, tag="msq")
                nc.vector.tensor_tensor(out=msq_t, in0=mean_t, in1=mean_t, op=Alu.mult)
                nc.vector.tensor_tensor(out=var_t, in0=var_t, in1=msq_t, op=Alu.subtract)
                # sd = sqrt(var + eps); rstd = 1/sd
                nc.scalar.activation(out=var_t, in_=var_t, func=Act.Sqrt,
                                     bias=eps_t, scale=1.0)
                rm_t = sb2.tile([1, 64], f32, tag="rm")
                nc.vector.reciprocal(out=rm_t[:, 0:32], in_=var_t)
                nc.vector.tensor_tensor(out=rm_t[:, 32:64], in0=mean_t,
                                        in1=rm_t[:, 0:32], op=Alu.mult)
                # broadcast across partitions: [128, 64]
                bc_ps = ps2.tile([128, 64], f32, tag="bc")
                nc.tensor.matmul(bc_ps, lhsT=ones1_t, rhs=rm_t, start=True, stop=True)
                nc.vector.tensor_copy(out=rmrb_t, in_=bc_ps)

                # G1Q/G1K[:, oc, t] = g1{q,k}[oc*128+p] * mr[t] - b{q,k}[oc*128+p]
                mr_ap = rm_t[:, 32:64]
                for g1t, bt, G1 in ((g1q_t, bq_t, G1Q_t), (g1k_t, bk_t, G1K_t)):
                    for oc in range(CC):
                        gq_ps = ps2.tile([128, T], f32, tag="gq", bufs=3)
                        nc.tensor.matmul(gq_ps, lhsT=g1t[:, 128 * oc:128 * (oc + 1)],
                                         rhs=mr_ap, start=True, stop=True)
                        nc.vector.tensor_scalar(
                            out=G1[:, oc, :], in0=gq_ps,
                            scalar1=bt[:, oc:oc + 1], scalar2=None,
                            op0=Alu.subtract)
                # CV[row, c] = mr[row % 32] * g1v[c] - bvb[row, c]
                mrpat_t = sb2.tile([1, 128], f32, tag="mrpat")
                mr_rep = bass.AP(tensor=rm_t.tensor, offset=rm_t.offset + 32,
                                 ap=[rm_t.ap[0], [0, 4], [1, 32]])
                nc.vector.tensor_copy(out=mrpat_t.rearrange("q (a b) -> q a b", a=4),
                                      in_=mr_rep)
                cv_ps = ps2.tile([128, C], f32, tag="cv")
                nc.tensor.matmul(cv_ps, lhsT=mrpat_t, rhs=g1v_t, start=True, stop=True)
                nc.vector.tensor_tensor(out=CV_t, in0=cv_ps, in1=bvb_t, op=Alu.subtract)

            # broadcast views: [128, 16(pix, step0), 32(t)]
            rbv = ap3(rmrb_t, 0, [[0, 16], [1, 32]])

            # ================= Phase 3: main =================
            with ExitStack() as p3:
                sb3 = p3.enter_context(tc.tile_pool(name="p3sb", bufs=1))
                mm_ps_pool = p3.enter_context(tc.tile_pool(name="mmps", bufs=3, space="PSUM"))
                s_ps_pool = p3.enter_context(tc.tile_pool(name="sps", bufs=3, space="PSUM"))
                po_ps_pool = p3.enter_context(tc.tile_pool(name="pops", bufs=2, space="PSUM"))

                for blk in range(NB):
                    # ---- load x block tiles [128, 32 t, 64 p]
                    x_t = []
                    for cc in range(CC):
                        xt = sb3.tile([128, T, PB], f32, tag="x", bufs=8)
                        nc.sync.dma_start(
                            out=xt,
                            in_=xv[cc * 128:(cc + 1) * 128, :, PB * blk:PB * (blk + 1)],
                        )
                        x_t.append(xt)

                    # per h-chunk state
                    for hc in range(4):
                        # ---- h = x*r - m*r   (pixel-major [128, 512] = 16 p x 32 t)
                        h_t = []
                        for cc in range(CC):
                            ht = sb3.tile([128, 512], f32r, tag="h", bufs=8)
                            h3 = ht.rearrange("q (p t) -> q p t", p=16)
                            xs = ap3(x_t[cc], 16 * hc, [[1, 16], [64, 32]])
                            nc.gpsimd.tensor_tensor(out=h3, in0=xs, in1=rbv, op=Alu.mult)
                            h_t.append(ht)

                        # ---- q, k GEMMs (output chunks [128, 512])
                        q_t, k_t = [], []
                        for wt, bt, dst in ((wq_t, bq_t, q_t), (wk_t, bk_t, k_t)):
                            for oc in range(CC):
                                mm_ps = mm_ps_pool.tile([128, 512], f32, tag="mm")
                                for cc in range(CC):
                                    nc.tensor.matmul(
                                        mm_ps,
                                        lhsT=wt[:, cc, 128 * oc:128 * (oc + 1)],
                                        rhs=h_t[cc],
                                        start=(cc == 0),
                                        stop=(cc == CC - 1),
                                    )
                                qt = sb3.tile([128, 512], f32r,
                                              tag=("q" if dst is q_t else "k"), bufs=8)
                                G1 = G1Q_t if dst is q_t else G1K_t
                                g1view = bass.AP(
                                    tensor=G1.tensor,
                                    offset=G1[:, oc, :].offset,
                                    ap=[G1.ap[0], [0, 16], [1, 32]])
                                nc.vector.scalar_tensor_tensor(
                                    out=qt.rearrange("q (p t) -> q p t", p=16),
                                    in0=mm_ps.rearrange("q (p t) -> q p t", p=16),
                                    scalar=1.0, in1=g1view,
                                    op0=Alu.mult, op1=Alu.subtract)
                                dst.append(qt)

                        # ---- vT per group (4 groups of 4 pixels in this h-chunk)
                        vt_t = []
                        for g in range(4):
                            mm_ps = mm_ps_pool.tile([128, 512], f32, tag="mm")
                            for cc in range(CC):
                                nc.tensor.matmul(
                                    mm_ps,
                                    lhsT=h_t[cc][:, 128 * g:128 * (g + 1)],
                                    rhs=wv_t[:, cc, :],
                                    start=(cc == 0),
                                    stop=(cc == CC - 1),
                                )
                            vt = sb3.tile([128, 512], bf16, tag="vt", bufs=6)
                            nc.vector.scalar_tensor_tensor(
                                out=vt, in0=mm_ps, scalar=1.0, in1=CV_t,
                                op0=Alu.mult, op1=Alu.subtract)
                            vt_t.append(vt)

                        # ---- attention, paired groups for 256-wide S rhs
                        out_big = sb3.tile([128, CC, 512], f32r, tag="out", bufs=2,
                                           name=f"out_{blk}_{hc}")
                        for gp in range(2):
                            gA, gB = 2 * gp, 2 * gp + 1
                            s_ps = {}
                            for g in (gA, gB):
                                sp = s_ps_pool.tile([128, 256], f32, tag="s")
                                for oc in range(CC):
                                    nc.tensor.matmul(
                                        sp,
                                        lhsT=q_t[oc][:, 128 * g:128 * (g + 1)],
                                        rhs=k_t[oc][:, 128 * gA:128 * gA + 256],
                                        start=(oc == 0),
                                        stop=(oc == CC - 1),
                                    )
                                s_ps[g] = sp
                            for g in (gA, gB):
                                half = g - gA
                                sm_t = sb3.tile([128, 128], f32, tag="sm", bufs=6)
                                nc.vector.tensor_tensor(
                                    out=sm_t, in0=s_ps[g][:, 128 * half:128 * (half + 1)],
                                    in1=mask_t, op=Alu.add)
                                p_t = sb3.tile([128, 128], bf16, tag="p", bufs=6)
                                rs_t = sb3.tile([128, 1], f32, tag="rs", bufs=4)
                                nc.scalar.activation(out=p_t, in_=sm_t, func=Act.Exp,
                                                     accum_out=rs_t)
                                ri_t = sb3.tile([128, 1], f32, tag="ri", bufs=4)
                                nc.vector.reciprocal(out=ri_t, in_=rs_t)
                                nc.vector.tensor_scalar_mul(out=p_t, in0=p_t, scalar1=ri_t)
                                pt_ps = po_ps_pool.tile([128, 128], bf16, tag="po")
                                nc.tensor.transpose(pt_ps, p_t, ident_t)
                                pt_t = sb3.tile([128, 128], bf16, tag="ptsb", bufs=6)
                                nc.scalar.copy(out=pt_t, in_=pt_ps)
                                out_ps = po_ps_pool.tile([128, CC, 128], f32, tag="po")
                                for cc in range(CC):
                                    nc.tensor.matmul(
                                        out_ps[:, cc, :],
                                        lhsT=vt_t[g][:, 128 * cc:128 * (cc + 1)],
                                        rhs=pt_t,
                                        start=True,
                                        stop=True,
                                    )
                                nc.vector.tensor_copy(
                                    out=out_big[:, :, 128 * g:128 * (g + 1)],
                                    in_=out_ps)

                        # ---- proj + residual into x tiles (in place)
                        for oc in range(CC):
                            mm_ps = mm_ps_pool.tile([128, 512], f32, tag="mm")
                            for cc in range(CC):
                                nc.tensor.matmul(
                                    mm_ps,
                                    lhsT=wp_t[:, cc, 128 * oc:128 * (oc + 1)],
                                    rhs=out_big[:, cc, :],
                                    start=(cc == 0),
                                    stop=(cc == CC - 1),
                                )
                            xres = ap3(x_t[oc], 16 * hc, [[1, 16], [64, 32]])
                            nc.vector.scalar_tensor_tensor(
                                out=xres,
                                in0=mm_ps.rearrange("q (p t) -> q p t", p=16),
                                scalar=1.0,
                                in1=xres,
                                op0=Alu.mult,
                                op1=Alu.add,
                            )

                    # ---- store block
                    for cc in range(CC):
                        nc.sync.dma_start(
                            out=yv[cc * 128:(cc + 1) * 128, :, PB * blk:PB * (blk + 1)],
                            in_=x_t[cc],
                        )

    nc.compile()
    return nc


def _host_prep(gamma, beta, wq, wk, wv, wproj):
    scale = float(C) ** -0.5
    g = gamma.astype(np.float64)
    b = beta.astype(np.float64)
    wq64 = wq.astype(np.float64)
    wk64 = wk.astype(np.float64)
    wv64 = wv.astype(np.float64)
    wqt = np.ascontiguousarray(((wq64 * g[None, :]) * scale).T.astype(np.float32))
    wkt = np.ascontiguousarray((wk64 * g[None, :]).T.astype(np.float32))
    wvt = np.ascontiguousarray((wv64 * g[None, :]).T.astype(np.float32))
    wpt = np.ascontiguousarray(wproj.astype(np.float32).T)
    bq = ((wq64 @ b) * scale).astype(np.float32)
    bk = (wk64 @ b).astype(np.float32)
    bv = (wv64 @ b).astype(np.float32)
    bvb = np.ascontiguousarray(np.broadcast_to(bv[None, :], (128, C)))
    g1q = np.ascontiguousarray(wqt.sum(axis=0, dtype=np.float64).astype(np.float32)[None, :])
    g1k = np.ascontiguousarray(wkt.sum(axis=0, dtype=np.float64).astype(np.float32)[None, :])
    g1v = np.ascontiguousarray(wvt.sum(axis=0, dtype=np.float64).astype(np.float32)[None, :])
    # additive causal/block-diag mask for [128 rows=(p,t), 128 cols=(p,s)]
    idx = np.arange(128)
    pi, ti = idx[:, None] // 32, idx[:, None] % 32
    pj, tj = idx[None, :] // 32, idx[None, :] % 32
    mask = np.where((pi == pj) & (tj <= ti), 0.0, -1e30).astype(np.float32)
    import ml_dtypes
    ident = np.eye(128, dtype=ml_dtypes.bfloat16)
    return wqt, wkt, wvt, wpt, bq, bk, bvb, mask, ident, g1q, g1k, g1v


def _get_runner():
    """Build (once) a sharded jitted executable for the compiled Bass module.

    Mirrors concourse.bass2jax.run_bass_via_pjrt's multi-core path, but keeps
    the jitted function so repeated calls don't retrace, and exposes enough
    structure for execution-only benchmarking.
    """
    if "runner" in _CACHE:
        return _CACHE["runner"]

    import jax
    from jax.sharding import Mesh, PartitionSpec
    from jax.experimental.shard_map import shard_map
    from concourse import bass2jax, mybir

    nc = _CACHE.get("nc")
    if nc is None:
        nc = _build()
        _CACHE["nc"] = nc

    bass2jax.install_neuronx_cc_hook()

    partition_name = nc.partition_id_tensor.name if nc.partition_id_tensor else None
    in_names, out_names, out_avals = [], [], []
    for alloc in nc.m.functions[0].allocations:
        if not isinstance(alloc, mybir.MemoryLocationSet):
            continue
        name = alloc.memorylocations[0].name
        if alloc.kind == "ExternalInput":
            if name != partition_name:
                in_names.append(name)
        elif alloc.kind == "ExternalOutput":
            out_names.append(name)
            shape = tuple(alloc.tensor_shape)
            dtype = mybir.dt.np(alloc.dtype)
            out_avals.append(jax.core.ShapedArray(shape, dtype))
    n_params = len(in_names)
    n_outs = len(out_avals)
    all_in_names = list(in_names) + list(out_names)
    if partition_name is not None:
        all_in_names.append(partition_name)
    donate = tuple(range(n_params, n_params + n_outs))

    def _body(*args):
        operands = list(args)
        if partition_name is not None:
            operands.append(bass2jax.partition_id_tensor())
        outs = bass2jax._bass_exec_p.bind(
            *operands,
            out_avals=tuple(out_avals),
            in_names=tuple(all_in_names),
            out_names=tuple(out_names),
            lowering_input_output_aliases=(),
            sim_require_finite=True,
            sim_require_nnan=True,
            nc=nc,
        )
        return tuple(outs)

    devices = jax.devices()[:8]
    mesh = Mesh(np.asarray(devices), ("core",))
    in_specs = (PartitionSpec("core"),) * (n_params + n_outs)
    out_specs = (PartitionSpec("core"),) * n_outs
    sharded = jax.jit(
        shard_map(_body, mesh=mesh, in_specs=in_specs, out_specs=out_specs,
                  check_rep=False),
        donate_argnums=donate,
        keep_unused=True,
    )
    runner = {
        "sharded": sharded,
        "mesh": mesh,
        "in_names": in_names,
        "out_names": out_names,
        "out_avals": out_avals,
    }
    _CACHE["runner"] = runner
    return runner


def _run(in_maps):
    runner = _get_runner()
    in_names = runner["in_names"]
    out_avals = runner["out_avals"]
    per_core = [[np.asarray(m[name]) for name in in_names] for m in in_maps]
    concat_in = [
        np.concatenate([per_core[c][i] for c in range(8)], axis=0)
        for i in range(len(in_names))
    ]
    concat_zeros = [
        np.zeros((8 * a.shape[0], *a.shape[1:]), a.dtype) for a in out_avals
    ]
    out_arrs = runner["sharded"](*concat_in, *concat_zeros)
    _CACHE["last_run"] = (concat_in, [tuple(a.shape) for a in out_avals])
    return [
        {
            name: np.asarray(out_arrs[i]).reshape(8, *out_avals[i].shape)[c]
            for i, name in enumerate(runner["out_names"])
        }
        for c in range(8)
    ]


def bench(iters=10):
    """Time device-side execution with inputs pre-staged on the devices."""
    import time
    import jax
    from jax.sharding import NamedSharding, PartitionSpec

    runner = _get_runner()
    concat_in, out_shapes = _CACHE["last_run"]
    sharding = NamedSharding(runner["mesh"], PartitionSpec("core"))
    dev_in = [jax.device_put(a, sharding) for a in concat_in]
    for a in dev_in:
        a.block_until_ready()
    times = []
    for _ in range(iters):
        zeros = [
            jax.device_put(np.zeros((8 * s[0], *s[1:]), np.float32), sharding)
            for s in out_shapes
        ]
        for z in zeros:
            z.block_until_ready()
        t0 = time.perf_counter()
        outs = runner["sharded"](*dev_in, *zeros)
        for o in outs:
            o.block_until_ready()
        t1 = time.perf_counter()
        times.append(t1 - t0)
    return times


def kernel(x, gamma, beta, wq, wk, wv, wproj):
    x = np.asarray(x, dtype=np.float32)
    gamma = np.asarray(gamma, dtype=np.float32)
    beta = np.asarray(beta, dtype=np.float32)
    wq = np.asarray(wq, dtype=np.float32)
    wk = np.asarray(wk, dtype=np.float32)
    wv = np.asarray(wv, dtype=np.float32)
    wproj = np.asarray(wproj, dtype=np.float32)

    wqt, wkt, wvt, wpt, bq, bk, bvb, mask, ident, g1q, g1k, g1v = _host_prep(
        gamma, beta, wq, wk, wv, wproj)

    B = x.shape[0]
    in_maps = []
    for i in range(8):
        b, h0 = i // 2, HL * (i % 2)
        in_maps.append({
            "x": np.ascontiguousarray(x[b, :, :, h0:h0 + HL, :]),
            "wqt": wqt, "wkt": wkt, "wvt": wvt, "wpt": wpt,
            "bq": bq, "bk": bk, "bvb": bvb, "mask": mask, "ident": ident,
            "ones": np.ones((128, 32), dtype=np.float32),
            "g1q": g1q, "g1k": g1k, "g1v": g1v,
        })

    results = _run(in_maps)

    y = np.empty((B, C, T, 2 * HL, W), dtype=np.float32)
    for i in range(8):
        b, h0 = i // 2, HL * (i % 2)
        y[b, :, :, h0:h0 + HL, :] = results[i]["y"]
    return y


# revision 23
# speedup vs baseline: 1.2029x; 1.2029x over previous
"""Causal temporal attention (CausalGroupNorm + per-pixel temporal attention)
on 8 Trainium2 NeuronCores.

Sharding: data-parallel over the B*H*W pixel pseudo-batch. Core i handles
batch b = i//2 and h-rows [16*(i%2), 16*(i%2)+16) -- 512 pixels per core,
each with a [C=512, T=32] temporal sequence. The CxC projection weights are
replicated (pre-transposed / gamma-folded on host).

Per-core pipeline (single Tile kernel):
  Phase 1: GroupNorm stats. Stream x tiles, colsum x and x^2 on the PE
           (lhsT = ones), per-t partial sums -> [1, 64] = [sum | sumsq].
  Phase 2: pairwise AllReduce of the [1, 64] stats (cores sharing a batch),
           derive rstd[t], mean[t]*rstd[t]; broadcast across partitions via
           a K=1 PE matmul.
  Phase 3: stream 8 blocks of 64 pixels:
           h = x*r - m*r (pixel-major columns) -> q,k GEMMs, vT per group,
           S = q^T k (fp32r, paired groups for 256-wide rhs), masked softmax
           (ACT exp + fused rowsum), P transpose on PE, out = vT^T @ Pt,
           proj GEMM + residual add into the x tile, DMA out.
"""

import sys
import os

sys.path.insert(0, "/opt/trn_rl_repo")

import numpy as np

C = 512
T = 32
HL = 16          # h-rows per core
W = 32
NL = HL * W      # pixels per core = 512
PB = 64          # pixels per block
NB = NL // PB    # 8 blocks
CC = C // 128    # 4 chunks
NELEM = float(C * 2 * NL)  # elements per (b,t) frame for the group norm (C*H*W)
EPS = 1e-6

_CACHE = {}


def _build(collective=True):
    from concourse import bacc, tile, mybir, bass

    f32 = mybir.dt.float32
    f32r = mybir.dt.float32r
    bf16 = mybir.dt.bfloat16
    Alu = mybir.AluOpType
    Act = mybir.ActivationFunctionType

    nc = bacc.Bacc("TRN2", target_bir_lowering=False, debug=False, num_devices=8)

    x_d = nc.dram_tensor("x", [C, T, HL, W], f32, kind="ExternalInput").ap()
    wqt_d = nc.dram_tensor("wqt", [C, C], f32, kind="ExternalInput").ap()
    wkt_d = nc.dram_tensor("wkt", [C, C], f32, kind="ExternalInput").ap()
    wvt_d = nc.dram_tensor("wvt", [C, C], f32, kind="ExternalInput").ap()
    wpt_d = nc.dram_tensor("wpt", [C, C], f32, kind="ExternalInput").ap()
    bq_d = nc.dram_tensor("bq", [C], f32, kind="ExternalInput").ap()
    bk_d = nc.dram_tensor("bk", [C], f32, kind="ExternalInput").ap()
    bvb_d = nc.dram_tensor("bvb", [128, C], f32, kind="ExternalInput").ap()
    mask_d = nc.dram_tensor("mask", [128, 128], f32, kind="ExternalInput").ap()
    ident_d = nc.dram_tensor("ident", [128, 128], mybir.dt.bfloat16, kind="ExternalInput").ap()
    ones_d = nc.dram_tensor("ones", [128, 32], f32, kind="ExternalInput").ap()
    g1q_d = nc.dram_tensor("g1q", [1, C], f32, kind="ExternalInput").ap()
    g1k_d = nc.dram_tensor("g1k", [1, C], f32, kind="ExternalInput").ap()
    g1v_d = nc.dram_tensor("g1v", [1, C], f32, kind="ExternalInput").ap()
    y_d = nc.dram_tensor("y", [C, T, HL, W], f32, kind="ExternalOutput").ap()

    xv = x_d.rearrange("c t h w -> c t (h w)")   # [512, 32, 512]
    yv = y_d.rearrange("c t h w -> c t (h w)")

    def ap3(base, off, dims):
        return bass.AP(tensor=base.tensor, offset=base.offset + off, ap=[base.ap[0]] + dims)

    with tile.TileContext(nc) as tc:
        from contextlib import ExitStack

        with ExitStack() as ctx:
            persist = ctx.enter_context(tc.tile_pool(name="persist", bufs=1))

            # ---- constants / weights
            wq_t = persist.tile([128, CC, C], f32r, tag="wq")
            nc.sync.dma_start(out=wq_t, in_=wqt_d.rearrange("(cc p) o -> p cc o", p=128).bitcast(f32r))
            wk_t = persist.tile([128, CC, C], f32r, tag="wk")
            nc.sync.dma_start(out=wk_t, in_=wkt_d.rearrange("(cc p) o -> p cc o", p=128).bitcast(f32r))
            wv_t = persist.tile([128, CC, C], f32r, tag="wv")
            nc.sync.dma_start(out=wv_t, in_=wvt_d.rearrange("(cc p) o -> p cc o", p=128).bitcast(f32r))
            wp_t = persist.tile([128, CC, C], f32r, tag="wp")
            nc.sync.dma_start(out=wp_t, in_=wpt_d.rearrange("(cc p) o -> p cc o", p=128).bitcast(f32r))
            bq_t = persist.tile([128, CC], f32, tag="bq")
            nc.sync.dma_start(out=bq_t, in_=bq_d.rearrange("(cc p) -> p cc", p=128))
            bk_t = persist.tile([128, CC], f32, tag="bk")
            nc.sync.dma_start(out=bk_t, in_=bk_d.rearrange("(cc p) -> p cc", p=128))
            bvb_t = persist.tile([128, C], f32, tag="bvb")
            nc.sync.dma_start(out=bvb_t, in_=bvb_d)
            mask_t = persist.tile([128, 128], f32, tag="mask")
            nc.sync.dma_start(out=mask_t, in_=mask_d)
            ident_t = persist.tile([128, 128], bf16, tag="ident")
            nc.sync.dma_start(out=ident_t, in_=ident_d)
            g1q_t = persist.tile([1, C], f32, tag="g1q")
            nc.sync.dma_start(out=g1q_t, in_=g1q_d)
            g1k_t = persist.tile([1, C], f32, tag="g1k")
            nc.sync.dma_start(out=g1k_t, in_=g1k_d)
            g1v_t = persist.tile([1, C], f32, tag="g1v")
            nc.sync.dma_start(out=g1v_t, in_=g1v_d)

            ones_t = persist.tile([128, 32], f32r, tag="ones")
            nc.sync.dma_start(out=ones_t, in_=ones_d.bitcast(f32r))
            ones1_t = persist.tile([1, 128], f32, tag="ones1")
            nc.vector.memset(ones1_t, 1.0)
            eps_t = persist.tile([1, 1], f32, tag="eps")
            nc.vector.memset(eps_t, EPS)

            # stats accumulators [32(dummy), 32 t] each, zeroed
            acc1_t = persist.tile([32, T], f32, tag="acc1")
            nc.vector.memset(acc1_t, 0.0)
            acc2_t = persist.tile([32, T], f32, tag="acc2")
            nc.vector.memset(acc2_t, 0.0)

            # persist broadcast of rstd | mean*rstd  [128, 64]
            rmrb_t = persist.tile([128, 64], f32, tag="rmrb")
            # per-(o,t) corrections: q/k evac subtracts outer(g1, mr) - bias
            G1Q_t = persist.tile([128, CC, T], f32, tag="G1Q")
            G1K_t = persist.tile([128, CC, T], f32, tag="G1K")
            # vT-row correction: outer(mr_rowpattern, g1v) - bvb  [128, 512]
            CV_t = persist.tile([128, C], f32, tag="CV")

            # ================= Phase 1: stats =================
            with ExitStack() as p1:
                sb1 = p1.enter_context(tc.tile_pool(name="p1sb", bufs=1))
                ps1 = p1.enter_context(tc.tile_pool(name="p1ps", bufs=6, space="PSUM"))

                P1B = 128
                for blk in range(NL // P1B):
                    x1 = []
                    for cc in range(CC):
                        x1_t = sb1.tile([128, T, P1B], f32r, tag="x1", bufs=8)
                        nc.sync.dma_start(
                            out=x1_t,
                            in_=xv[cc * 128:(cc + 1) * 128, :, P1B * blk:P1B * (blk + 1)].bitcast(f32r),
                        )
                        x1.append(x1_t.rearrange("p t w -> p (t w)"))
                    for ns in range(T * P1B // 512):
                        sl = slice(512 * ns, 512 * (ns + 1))
                        cs_ps = ps1.tile([32, 512], f32, tag="cs")
                        sq_ps = ps1.tile([32, 512], f32, tag="cs")
                        for cc in range(CC):
                            nc.tensor.matmul(
                                cs_ps, lhsT=ones_t, rhs=x1[cc][:, sl],
                                start=(cc == 0), stop=(cc == CC - 1))
                        for cc in range(CC):
                            sq_t = sb1.tile([128, 512], f32r, tag="xsq", bufs=4)
                            nc.scalar.activation(out=sq_t, in_=x1[cc][:, sl],
                                                 func=Act.Square)
                            nc.tensor.matmul(
                                sq_ps, lhsT=ones_t, rhs=sq_t,
                                start=(cc == 0), stop=(cc == CC - 1))
                        for ps, acc in ((cs_ps, acc1_t), (sq_ps, acc2_t)):
                            red_t = sb1.tile([32, 4], f32, tag="red", bufs=4)
                            nc.vector.reduce_sum(
                                out=red_t,
                                in_=ps.rearrange("p (t w) -> p t w", t=4),
                                axis=mybir.AxisListType.X,
                            )
                            nc.vector.tensor_tensor(
                                out=acc[:, 4 * ns:4 * (ns + 1)],
                                in0=acc[:, 4 * ns:4 * (ns + 1)],
                                in1=red_t,
                                op=Alu.add,
                            )

            # ================= Phase 2: allreduce + derive =================
            with ExitStack() as p2:
                sb2 = p2.enter_context(tc.tile_pool(name="p2sb", bufs=1))
                ps2 = p2.enter_context(tc.tile_pool(name="p2ps", bufs=1, space="PSUM"))
                dram = p2.enter_context(tc.tile_pool(name="p2dram", bufs=1, space="DRAM"))

                stats_t = sb2.tile([1, 64], f32, tag="stats")
                nc.vector.tensor_copy(out=stats_t[:, 0:32], in_=acc1_t[0:1, :])
                nc.vector.tensor_copy(out=stats_t[:, 32:64], in_=acc2_t[0:1, :])

                st_in = dram.tile([1, 64], f32)
                st_out = dram.tile([1, 64], f32)
                nc.gpsimd.dma_start(out=st_in, in_=stats_t)
                if collective:
                    nc.gpsimd.collective_compute(
                        "AllReduce",
                        Alu.add,
                        replica_groups=[[0, 1], [2, 3], [4, 5], [6, 7]],
                        ins=[st_in.opt()],
                        outs=[st_out.opt()],
                    )
                else:
                    nc.gpsimd.dma_start(out=st_out, in_=st_in)
                vr_t = sb2.tile([1, 64], f32, tag="vr")
                nc.gpsimd.dma_start(out=vr_t, in_=st_out)

                # mean = S1/N ; e2 = S2/N ; var = e2 - mean^2
                # rm_t = [ rstd(32) | mean*rstd(32) ]
                mean_t = sb2.tile([1, 32], f32, tag="mean")
                nc.scalar.mul(out=mean_t, in_=vr_t[:, 0:32], mul=1.0 / NELEM)
                var_t = sb2.tile([1, 32], f32, tag="var")
                nc.scalar.mul(out=var_t, in_=vr_t[:, 32:64], mul=1.0 / NELEM)
                msq_t = sb2.tile([1, 32], f32, tag="msq")
                nc.vector.tensor_tensor(out=msq_t, in0=mean_t, in1=mean_t, op=Alu.mult)
                nc.vector.tensor_tensor(out=var_t, in0=var_t, in1=msq_t, op=Alu.subtract)
                # sd = sqrt(var + eps); rstd = 1/sd
                nc.scalar.activation(out=var_t, in_=var_t, func=Act.Sqrt,
                                     bias=eps_t, scale=1.0)
                rm_t = sb2.tile([1, 64], f32, tag="rm")
                nc.vector.reciprocal(out=rm_t[:, 0:32], in_=var_t)
                nc.vector.tensor_tensor(out=rm_t[:, 32:64], in0=mean_t,
                                        in1=rm_t[:, 0:32], op=Alu.mult)
                # broadcast across partitions: [128, 64]
                bc_ps = ps2.tile([128, 64], f32, tag="bc")
                nc.tensor.matmul(bc_ps, lhsT=ones1_t, rhs=rm_t, start=True, stop=True)
                nc.vector.tensor_copy(out=rmrb_t, in_=bc_ps)

                # G1Q/G1K[:, oc, t] = g1{q,k}[oc*128+p] * mr[t] - b{q,k}[oc*128+p]
                mr_ap = rm_t[:, 32:64]
                for g1t, bt, G1 in ((g1q_t, bq_t, G1Q_t), (g1k_t, bk_t, G1K_t)):
                    for oc in range(CC):
                        gq_ps = ps2.tile([128, T], f32, tag="gq", bufs=3)
                        nc.tensor.matmul(gq_ps, lhsT=g1t[:, 128 * oc:128 * (oc + 1)],
                                         rhs=mr_ap, start=True, stop=True)
                        nc.vector.tensor_scalar(
                            out=G1[:, oc, :], in0=gq_ps,
                            scalar1=bt[:, oc:oc + 1], scalar2=None,
                            op0=Alu.subtract)
                # CV[row, c] = mr[row % 32] * g1v[c] - bvb[row, c]
                mrpat_t = sb2.tile([1, 128], f32, tag="mrpat")
                mr_rep = bass.AP(tensor=rm_t.tensor, offset=rm_t.offset + 32,
                                 ap=[rm_t.ap[0], [0, 4], [1, 32]])
                nc.vector.tensor_copy(out=mrpat_t.rearrange("q (a b) -> q a b", a=4),
                                      in_=mr_rep)
                cv_ps = ps2.tile([128, C], f32, tag="cv")
                nc.tensor.matmul(cv_ps, lhsT=mrpat_t, rhs=g1v_t, start=True, stop=True)
                nc.vector.tensor_tensor(out=CV_t, in0=cv_ps, in1=bvb_t, op=Alu.subtract)

            # broadcast views: [128, 16(pix, step0), 32(t)]
            rbv = ap3(rmrb_t, 0, [[0, 16], [1, 32]])

            # ================= Phase 3: main =================
            with ExitStack() as p3:
                sb3 = p3.enter_context(tc.tile_pool(name="p3sb", bufs=1))
                mm_ps_pool = p3.enter_context(tc.tile_pool(name="mmps", bufs=6, space="PSUM"))

                po_ps_pool = p3.enter_context(tc.tile_pool(name="pops", bufs=2, space="PSUM"))

                for blk in range(NB):
                    # ---- load x block tiles [128, 32 t, 64 p]
                    x_t = []
                    for cc in range(CC):
                        xt = sb3.tile([128, T, PB], f32, tag="x", bufs=8)
                        nc.sync.dma_start(
                            out=xt,
                            in_=xv[cc * 128:(cc + 1) * 128, :, PB * blk:PB * (blk + 1)],
                        )
                        x_t.append(xt)

                    # per h-chunk state
                    for hc in range(4):
                        # ---- h = x*r - m*r   (pixel-major [128, 512] = 16 p x 32 t)
                        h_t = []
                        for cc in range(CC):
                            ht = sb3.tile([128, 512], f32r, tag="h", bufs=8)
                            h3 = ht.rearrange("q (p t) -> q p t", p=16)
                            xs = ap3(x_t[cc], 16 * hc, [[1, 16], [64, 32]])
                            nc.gpsimd.tensor_tensor(out=h3, in0=xs, in1=rbv, op=Alu.mult)
                            h_t.append(ht)

                        # ---- q, k GEMMs (output chunks [128, 512])
                        q_t, k_t = [], []
                        for wt, bt, dst in ((wq_t, bq_t, q_t), (wk_t, bk_t, k_t)):
                            for oc in range(CC):
                                mm_ps = mm_ps_pool.tile([128, 512], f32, tag="mm")
                                for cc in range(CC):
                                    nc.tensor.matmul(
                                        mm_ps,
                                        lhsT=wt[:, cc, 128 * oc:128 * (oc + 1)],
                                        rhs=h_t[cc],
                                        start=(cc == 0),
                                        stop=(cc == CC - 1),
                                    )
                                qt = sb3.tile([128, 512], f32r,
                                              tag=("q" if dst is q_t else "k"), bufs=8)
                                G1 = G1Q_t if dst is q_t else G1K_t
                                g1view = bass.AP(
                                    tensor=G1.tensor,
                                    offset=G1[:, oc, :].offset,
                                    ap=[G1.ap[0], [0, 16], [1, 32]])
                                nc.vector.scalar_tensor_tensor(
                                    out=qt.rearrange("q (p t) -> q p t", p=16),
                                    in0=mm_ps.rearrange("q (p t) -> q p t", p=16),
                                    scalar=1.0, in1=g1view,
                                    op0=Alu.mult, op1=Alu.subtract)
                                dst.append(qt)

                        # ---- vT per group (4 groups of 4 pixels in this h-chunk)
                        vt_t = []
                        for g in range(4):
                            mm_ps = mm_ps_pool.tile([128, 512], f32, tag="mm")
                            for cc in range(CC):
                                nc.tensor.matmul(
                                    mm_ps,
                                    lhsT=h_t[cc][:, 128 * g:128 * (g + 1)],
                                    rhs=wv_t[:, cc, :],
                                    start=(cc == 0),
                                    stop=(cc == CC - 1),
                                )
                            vt = sb3.tile([128, 512], bf16, tag="vt", bufs=6)
                            nc.vector.scalar_tensor_tensor(
                                out=vt, in0=mm_ps, scalar=1.0, in1=CV_t,
                                op0=Alu.mult, op1=Alu.subtract)
                            vt_t.append(vt)

                        # ---- attention, paired groups for 256-wide S rhs
                        out_big = sb3.tile([128, CC, 512], f32r, tag="out", bufs=2,
                                           name=f"out_{blk}_{hc}")
                        for gp in range(2):
                            gA, gB = 2 * gp, 2 * gp + 1
                            s_ps = {}
                            for g in (gA, gB):
                                sp = mm_ps_pool.tile([128, 256], f32, tag="mm")
                                for oc in range(CC):
                                    nc.tensor.matmul(
                                        sp,
                                        lhsT=q_t[oc][:, 128 * g:128 * (g + 1)],
                                        rhs=k_t[oc][:, 128 * gA:128 * gA + 256],
                                        start=(oc == 0),
                                        stop=(oc == CC - 1),
                                    )
                                s_ps[g] = sp
                            for g in (gA, gB):
                                half = g - gA
                                sm_t = sb3.tile([128, 128], f32, tag="sm", bufs=6)
                                nc.vector.tensor_tensor(
                                    out=sm_t, in0=s_ps[g][:, 128 * half:128 * (half + 1)],
                                    in1=mask_t, op=Alu.add)
                                p_t = sb3.tile([128, 128], bf16, tag="p", bufs=6)
                                rs_t = sb3.tile([128, 1], f32, tag="rs", bufs=4)
                                nc.scalar.activation(out=p_t, in_=sm_t, func=Act.Exp,
                                                     accum_out=rs_t)
                                ri_t = sb3.tile([128, 1], f32, tag="ri", bufs=4)
                                nc.vector.reciprocal(out=ri_t, in_=rs_t)
                                nc.vector.tensor_scalar_mul(out=p_t, in0=p_t, scalar1=ri_t)
                                pt_ps = po_ps_pool.tile([128, 128], bf16, tag="po")
                                nc.tensor.transpose(pt_ps, p_t, ident_t)
                                pt_t = sb3.tile([128, 128], bf16, tag="ptsb", bufs=6)
                                nc.scalar.copy(out=pt_t, in_=pt_ps)
                                out_ps = po_ps_pool.tile([128, CC, 128], f32, tag="po")
                                for cc in range(CC):
                                    nc.tensor.matmul(
                                        out_ps[:, cc, :],
                                        lhsT=vt_t[g][:, 128 * cc:128 * (cc + 1)],
                                        rhs=pt_t,
                                        start=True,
                                        stop=True,
                                    )
                                nc.vector.tensor_copy(
                                    out=out_big[:, :, 128 * g:128 * (g + 1)],
                                    in_=out_ps)

                        # ---- proj + residual into x tiles (in place)
                        for oc in range(CC):
                            mm_ps = mm_ps_pool.tile([128, 512], f32, tag="mm")
                            for cc in range(CC):
                                nc.tensor.matmul(
                                    mm_ps,
                                    lhsT=wp_t[:, cc, 128 * oc:128 * (oc + 1)],
                                    rhs=out_big[:, cc, :],
                                    start=(cc == 0),
                                    stop=(cc == CC - 1),
                                )
                            xres = ap3(x_t[oc], 16 * hc, [[1, 16], [64, 32]])
                            nc.vector.scalar_tensor_tensor(
                                out=xres,
                                in0=mm_ps.rearrange("q (p t) -> q p t", p=16),
                                scalar=1.0,
                                in1=xres,
                                op0=Alu.mult,
                                op1=Alu.add,
                            )

                    # ---- store block
                    for cc in range(CC):
                        nc.sync.dma_start(
                            out=yv[cc * 128:(cc + 1) * 128, :, PB * blk:PB * (blk + 1)],
                            in_=x_t[cc],
                        )

    nc.compile()
    return nc


def _host_prep(gamma, beta, wq, wk, wv, wproj):
    scale = float(C) ** -0.5
    g = gamma.astype(np.float64)
    b = beta.astype(np.float64)
    wq64 = wq.astype(np.float64)
    wk64 = wk.astype(np.float64)
    wv64 = wv.astype(np.float64)
    wqt = np.ascontiguousarray(((wq64 * g[None, :]) * scale).T.astype(np.float32))
    wkt = np.ascontiguousarray((wk64 * g[None, :]).T.astype(np.float32))
    wvt = np.ascontiguousarray((wv64 * g[None, :]).T.astype(np.float32))
    wpt = np.ascontiguousarray(wproj.astype(np.float32).T)
    bq = ((wq64 @ b) * scale).astype(np.float32)
    bk = (wk64 @ b).astype(np.float32)
    bv = (wv64 @ b).astype(np.float32)
    bvb = np.ascontiguousarray(np.broadcast_to(bv[None, :], (128, C)))
    g1q = np.ascontiguousarray(wqt.sum(axis=0, dtype=np.float64).astype(np.float32)[None, :])
    g1k = np.ascontiguousarray(wkt.sum(axis=0, dtype=np.float64).astype(np.float32)[None, :])
    g1v = np.ascontiguousarray(wvt.sum(axis=0, dtype=np.float64).astype(np.float32)[None, :])
    # additive causal/block-diag mask for [128 rows=(p,t), 128 cols=(p,s)]
    idx = np.arange(128)
    pi, ti = idx[:, None] // 32, idx[:, None] % 32
    pj, tj = idx[None, :] // 32, idx[None, :] % 32
    mask = np.where((pi == pj) & (tj <= ti), 0.0, -1e30).astype(np.float32)
    import ml_dtypes
    ident = np.eye(128, dtype=ml_dtypes.bfloat16)
    return wqt, wkt, wvt, wpt, bq, bk, bvb, mask, ident, g1q, g1k, g1v


def _get_runner():
    """Build (once) a sharded jitted executable for the compiled Bass module.

    Mirrors concourse.bass2jax.run_bass_via_pjrt's multi-core path, but keeps
    the jitted function so repeated calls don't retrace, and exposes enough
    structure for execution-only benchmarking.
    """
    if "runner" in _CACHE:
        return _CACHE["runner"]

    import jax
    from jax.sharding import Mesh, PartitionSpec
    from jax.experimental.shard_map import shard_map
    from concourse import bass2jax, mybir

    nc = _CACHE.get("nc")
    if nc is None:
        nc = _build()
        _CACHE["nc"] = nc

    bass2jax.install_neuronx_cc_hook()

    partition_name = nc.partition_id_tensor.name if nc.partition_id_tensor else None
    in_names, out_names, out_avals = [], [], []
    for alloc in nc.m.functions[0].allocations:
        if not isinstance(alloc, mybir.MemoryLocationSet):
            continue
        name = alloc.memorylocations[0].name
        if alloc.kind == "ExternalInput":
            if name != partition_name:
                in_names.append(name)
        elif alloc.kind == "ExternalOutput":
            out_names.append(name)
            shape = tuple(alloc.tensor_shape)
            dtype = mybir.dt.np(alloc.dtype)
            out_avals.append(jax.core.ShapedArray(shape, dtype))
    n_params = len(in_names)
    n_outs = len(out_avals)
    all_in_names = list(in_names) + list(out_names)
    if partition_name is not None:
        all_in_names.append(partition_name)
    donate = tuple(range(n_params, n_params + n_outs))

    def _body(*args):
        operands = list(args)
        if partition_name is not None:
            operands.append(bass2jax.partition_id_tensor())
        outs = bass2jax._bass_exec_p.bind(
            *operands,
            out_avals=tuple(out_avals),
            in_names=tuple(all_in_names),
            out_names=tuple(out_names),
            lowering_input_output_aliases=(),
            sim_require_finite=True,
            sim_require_nnan=True,
            nc=nc,
        )
        return tuple(outs)

    devices = jax.devices()[:8]
    mesh = Mesh(np.asarray(devices), ("core",))
    in_specs = (PartitionSpec("core"),) * (n_params + n_outs)
    out_specs = (PartitionSpec("core"),) * n_outs
    sharded = jax.jit(
        shard_map(_body, mesh=mesh, in_specs=in_specs, out_specs=out_specs,
                  check_rep=False),
        donate_argnums=donate,
        keep_unused=True,
    )
    runner = {
        "sharded": sharded,
        "mesh": mesh,
        "in_names": in_names,
        "out_names": out_names,
        "out_avals": out_avals,
    }
    _CACHE["runner"] = runner
    return runner


def _run(in_maps):
    runner = _get_runner()
    in_names = runner["in_names"]
    out_avals = runner["out_avals"]
    per_core = [[np.asarray(m[name]) for name in in_names] for m in in_maps]
    concat_in = [
        np.concatenate([per_core[c][i] for c in range(8)], axis=0)
        for i in range(len(in_names))
    ]
    concat_zeros = [
        np.zeros((8 * a.shape[0], *a.shape[1:]), a.dtype) for a in out_avals
    ]
    out_arrs = runner["sharded"](*concat_in, *concat_zeros)
    _CACHE["last_run"] = (concat_in, [tuple(a.shape) for a in out_avals])
    return [
        {
            name: np.asarray(out_arrs[i]).reshape(8, *out_avals[i].shape)[c]
            for i, name in enumerate(runner["out_names"])
        }
        for c in range(8)
    ]


def bench(iters=10):
    """Time device-side execution with inputs pre-staged on the devices."""
    import time
    import jax
    from jax.sharding import NamedSharding, PartitionSpec

    runner = _get_runner()
    concat_in, out_shapes = _CACHE["last_run"]
    sharding = NamedSharding(runner["mesh"], PartitionSpec("core"))
    dev_in = [jax.device_put(a, sharding) for a in concat_in]
    for a in dev_in:
        a.block_until_ready()
    times = []
    for _ in range(iters):
        zeros = [
            jax.device_put(np.zeros((8 * s[0], *s[1:]), np.float32), sharding)
            for s in out_shapes
        ]
        for z in zeros:
            z.block_until_ready()
        t0 = time.perf_counter()
        outs = runner["sharded"](*dev_in, *zeros)
        for o in outs:
            o.block_until_ready()
        t1 = time.perf_counter()
        times.append(t1 - t0)
    return times


def kernel(x, gamma, beta, wq, wk, wv, wproj):
    x = np.asarray(x, dtype=np.float32)
    gamma = np.asarray(gamma, dtype=np.float32)
    beta = np.asarray(beta, dtype=np.float32)
    wq = np.asarray(wq, dtype=np.float32)
    wk = np.asarray(wk, dtype=np.float32)
    wv = np.asarray(wv, dtype=np.float32)
    wproj = np.asarray(wproj, dtype=np.float32)

    wqt, wkt, wvt, wpt, bq, bk, bvb, mask, ident, g1q, g1k, g1v = _host_prep(
        gamma, beta, wq, wk, wv, wproj)

    B = x.shape[0]
    in_maps = []
    for i in range(8):
        b, h0 = i // 2, HL * (i % 2)
        in_maps.append({
            "x": np.ascontiguousarray(x[b, :, :, h0:h0 + HL, :]),
            "wqt": wqt, "wkt": wkt, "wvt": wvt, "wpt": wpt,
            "bq": bq, "bk": bk, "bvb": bvb, "mask": mask, "ident": ident,
            "ones": np.ones((128, 32), dtype=np.float32),
            "g1q": g1q, "g1k": g1k, "g1v": g1v,
        })

    results = _run(in_maps)

    y = np.empty((B, C, T, 2 * HL, W), dtype=np.float32)
    for i in range(8):
        b, h0 = i // 2, HL * (i % 2)
        y[b, :, :, h0:h0 + HL, :] = results[i]["y"]
    return y


# revision 26
# speedup vs baseline: 112.3268x; 93.3761x over previous
"""Causal temporal attention (CausalGroupNorm + per-pixel temporal attention)
on 8 Trainium2 NeuronCores.

Sharding: data-parallel over the B*H*W pixel pseudo-batch. Core i handles
batch b = i//2 and h-rows [16*(i%2), 16*(i%2)+16) -- 512 pixels per core,
each a [C=512, T=32] temporal sequence. CxC projection weights are
replicated (pre-transposed, gamma/scale-folded on host).

Per-core pipeline (one Tile kernel):
  Phase 1: GroupNorm stats. Stream x in [128c, 32t, 128pix] tiles; x^2 on
           ACT; per-(t) partial sums via PE ones-matmul column sums + DVE
           pixel reduction -> [1, 64] = [sum | sumsq].
  Phase 2: pairwise AllReduce of stats (the two cores sharing a batch),
           derive rstd/mean*rstd, build the rank-1 correction tiles
           (G1Q/G1K per o-chunk, CV for vT rows) and the partition
           broadcast of rstd via K=1 PE matmuls.
  Phase 3: stream 8 blocks of 64 pixels:
           h = x*rstd reordered to pixel-major columns (GpSimd);
           q,k GEMMs (fp32r) with fused mean/bias correction in the DVE
           evacuation; per-group vT GEMMs; S = q^T k with paired groups
           (256-wide moving operand keeps fp32r at full rate); masked
           softmax (DVE mask-add, ACT exp with fused row-sum); P transpose
           on PE (bf16); out = vT^T @ Pt in bf16; proj GEMM (fp32r) with
           residual added from the x tile in the evacuation; store.
"""

import sys
import os

sys.path.insert(0, "/opt/trn_rl_repo")

import numpy as np

C = 512
T = 32
HL = 16          # h-rows per core
W = 32
NL = HL * W      # pixels per core = 512
PB = 64          # pixels per block
NB = NL // PB    # 8 blocks
CC = C // 128    # 4 chunks
NELEM = float(C * 2 * NL)  # elements per (b,t) frame for the group norm (C*H*W)
EPS = 1e-6

_CACHE = {}


def _build(collective=True):
    from concourse import bacc, tile, mybir, bass

    f32 = mybir.dt.float32
    f32r = mybir.dt.float32r
    bf16 = mybir.dt.bfloat16
    Alu = mybir.AluOpType
    Act = mybir.ActivationFunctionType

    nc = bacc.Bacc("TRN2", target_bir_lowering=False, debug=False, num_devices=8)

    x_d = nc.dram_tensor("x", [C, T, HL, W], f32, kind="ExternalInput").ap()
    wqt_d = nc.dram_tensor("wqt", [C, C], f32, kind="ExternalInput").ap()
    wkt_d = nc.dram_tensor("wkt", [C, C], f32, kind="ExternalInput").ap()
    wvt_d = nc.dram_tensor("wvt", [C, C], f32, kind="ExternalInput").ap()
    wpt_d = nc.dram_tensor("wpt", [C, C], f32, kind="ExternalInput").ap()
    bq_d = nc.dram_tensor("bq", [C], f32, kind="ExternalInput").ap()
    bk_d = nc.dram_tensor("bk", [C], f32, kind="ExternalInput").ap()
    bvb_d = nc.dram_tensor("bvb", [128, C], f32, kind="ExternalInput").ap()
    mask_d = nc.dram_tensor("mask", [128, 128], f32, kind="ExternalInput").ap()
    ident_d = nc.dram_tensor("ident", [128, 128], mybir.dt.bfloat16, kind="ExternalInput").ap()
    ones_d = nc.dram_tensor("ones", [128, 32], f32, kind="ExternalInput").ap()
    g1q_d = nc.dram_tensor("g1q", [1, C], f32, kind="ExternalInput").ap()
    g1k_d = nc.dram_tensor("g1k", [1, C], f32, kind="ExternalInput").ap()
    g1v_d = nc.dram_tensor("g1v", [1, C], f32, kind="ExternalInput").ap()
    y_d = nc.dram_tensor("y", [C, T, HL, W], f32, kind="ExternalOutput").ap()

    xv = x_d.rearrange("c t h w -> c t (h w)")   # [512, 32, 512]
    yv = y_d.rearrange("c t h w -> c t (h w)")

    def ap3(base, off, dims):
        return bass.AP(tensor=base.tensor, offset=base.offset + off, ap=[base.ap[0]] + dims)

    with tile.TileContext(nc) as tc:
        from contextlib import ExitStack

        with ExitStack() as ctx:
            persist = ctx.enter_context(tc.tile_pool(name="persist", bufs=1))

            # ---- constants / weights
            wq_t = persist.tile([128, CC, C], f32r, tag="wq")
            nc.sync.dma_start(out=wq_t, in_=wqt_d.rearrange("(cc p) o -> p cc o", p=128).bitcast(f32r))
            wk_t = persist.tile([128, CC, C], f32r, tag="wk")
            nc.sync.dma_start(out=wk_t, in_=wkt_d.rearrange("(cc p) o -> p cc o", p=128).bitcast(f32r))
            wv_t = persist.tile([128, CC, C], f32r, tag="wv")
            nc.sync.dma_start(out=wv_t, in_=wvt_d.rearrange("(cc p) o -> p cc o", p=128).bitcast(f32r))
            wp_t = persist.tile([128, CC, C], f32r, tag="wp")
            nc.sync.dma_start(out=wp_t, in_=wpt_d.rearrange("(cc p) o -> p cc o", p=128).bitcast(f32r))
            bq_t = persist.tile([128, CC], f32, tag="bq")
            nc.sync.dma_start(out=bq_t, in_=bq_d.rearrange("(cc p) -> p cc", p=128))
            bk_t = persist.tile([128, CC], f32, tag="bk")
            nc.sync.dma_start(out=bk_t, in_=bk_d.rearrange("(cc p) -> p cc", p=128))
            bvb_t = persist.tile([128, C], f32, tag="bvb")
            nc.sync.dma_start(out=bvb_t, in_=bvb_d)
            mask_t = persist.tile([128, 128], f32, tag="mask")
            nc.sync.dma_start(out=mask_t, in_=mask_d)
            ident_t = persist.tile([128, 128], bf16, tag="ident")
            nc.sync.dma_start(out=ident_t, in_=ident_d)
            g1q_t = persist.tile([1, C], f32, tag="g1q")
            nc.sync.dma_start(out=g1q_t, in_=g1q_d)
            g1k_t = persist.tile([1, C], f32, tag="g1k")
            nc.sync.dma_start(out=g1k_t, in_=g1k_d)
            g1v_t = persist.tile([1, C], f32, tag="g1v")
            nc.sync.dma_start(out=g1v_t, in_=g1v_d)

            ones_t = persist.tile([128, 32], f32r, tag="ones")
            nc.sync.dma_start(out=ones_t, in_=ones_d.bitcast(f32r))
            ones1_t = persist.tile([1, 128], f32, tag="ones1")
            nc.vector.memset(ones1_t, 1.0)
            eps_t = persist.tile([1, 1], f32, tag="eps")
            nc.vector.memset(eps_t, EPS)

            # stats accumulators [32(dummy), 32 t] each, zeroed
            acc1_t = persist.tile([32, T], f32, tag="acc1")
            nc.vector.memset(acc1_t, 0.0)
            acc2_t = persist.tile([32, T], f32, tag="acc2")
            nc.vector.memset(acc2_t, 0.0)

            # persist broadcast of rstd | mean*rstd  [128, 64]
            rmrb_t = persist.tile([128, 64], f32, tag="rmrb")
            # per-(o,t) corrections: q/k evac subtracts outer(g1, mr) - bias
            G1Q_t = persist.tile([128, CC, T], f32, tag="G1Q")
            G1K_t = persist.tile([128, CC, T], f32, tag="G1K")
            # vT-row correction: outer(mr_rowpattern, g1v) - bvb  [128, 512]
            CV_t = persist.tile([128, C], f32, tag="CV")

            # ================= Phase 1: stats =================
            with ExitStack() as p1:
                sb1 = p1.enter_context(tc.tile_pool(name="p1sb", bufs=1))
                ps1 = p1.enter_context(tc.tile_pool(name="p1ps", bufs=6, space="PSUM"))

                P1B = 128
                for blk in range(NL // P1B):
                    x1 = []
                    for cc in range(CC):
                        x1_t = sb1.tile([128, T, P1B], f32r, tag="x1", bufs=8)
                        nc.sync.dma_start(
                            out=x1_t,
                            in_=xv[cc * 128:(cc + 1) * 128, :, P1B * blk:P1B * (blk + 1)].bitcast(f32r),
                        )
                        x1.append(x1_t.rearrange("p t w -> p (t w)"))
                    for ns in range(T * P1B // 512):
                        sl = slice(512 * ns, 512 * (ns + 1))
                        cs_ps = ps1.tile([32, 512], f32, tag="cs")
                        sq_ps = ps1.tile([32, 512], f32, tag="cs")
                        for cc in range(CC):
                            nc.tensor.matmul(
                                cs_ps, lhsT=ones_t, rhs=x1[cc][:, sl],
                                start=(cc == 0), stop=(cc == CC - 1))
                        for cc in range(CC):
                            sq_t = sb1.tile([128, 512], f32r, tag="xsq", bufs=4)
                            nc.scalar.activation(out=sq_t, in_=x1[cc][:, sl],
                                                 func=Act.Square)
                            nc.tensor.matmul(
                                sq_ps, lhsT=ones_t, rhs=sq_t,
                                start=(cc == 0), stop=(cc == CC - 1))
                        for ps, acc in ((cs_ps, acc1_t), (sq_ps, acc2_t)):
                            red_t = sb1.tile([32, 4], f32, tag="red", bufs=4)
                            nc.vector.reduce_sum(
                                out=red_t,
                                in_=ps.rearrange("p (t w) -> p t w", t=4),
                                axis=mybir.AxisListType.X,
                            )
                            nc.vector.tensor_tensor(
                                out=acc[:, 4 * ns:4 * (ns + 1)],
                                in0=acc[:, 4 * ns:4 * (ns + 1)],
                                in1=red_t,
                                op=Alu.add,
                            )

            # ================= Phase 2: allreduce + derive =================
            with ExitStack() as p2:
                sb2 = p2.enter_context(tc.tile_pool(name="p2sb", bufs=1))
                ps2 = p2.enter_context(tc.tile_pool(name="p2ps", bufs=1, space="PSUM"))
                dram = p2.enter_context(tc.tile_pool(name="p2dram", bufs=1, space="DRAM"))

                stats_t = sb2.tile([1, 64], f32, tag="stats")
                nc.vector.tensor_copy(out=stats_t[:, 0:32], in_=acc1_t[0:1, :])
                nc.vector.tensor_copy(out=stats_t[:, 32:64], in_=acc2_t[0:1, :])

                st_in = dram.tile([1, 64], f32)
                st_out = dram.tile([1, 64], f32)
                nc.gpsimd.dma_start(out=st_in, in_=stats_t)
                if collective:
                    nc.gpsimd.collective_compute(
                        "AllReduce",
                        Alu.add,
                        replica_groups=[[0, 1], [2, 3], [4, 5], [6, 7]],
                        ins=[st_in.opt()],
                        outs=[st_out.opt()],
                    )
                else:
                    nc.gpsimd.dma_start(out=st_out, in_=st_in)
                vr_t = sb2.tile([1, 64], f32, tag="vr")
                nc.gpsimd.dma_start(out=vr_t, in_=st_out)

                # mean = S1/N ; e2 = S2/N ; var = e2 - mean^2
                # rm_t = [ rstd(32) | mean*rstd(32) ]
                mean_t = sb2.tile([1, 32], f32, tag="mean")
                nc.scalar.mul(out=mean_t, in_=vr_t[:, 0:32], mul=1.0 / NELEM)
                var_t = sb2.tile([1, 32], f32, tag="var")
                nc.scalar.mul(out=var_t, in_=vr_t[:, 32:64], mul=1.0 / NELEM)
                msq_t = sb2.tile([1, 32], f32, tag="msq")
                nc.vector.tensor_tensor(out=msq_t, in0=mean_t, in1=mean_t, op=Alu.mult)
                nc.vector.tensor_tensor(out=var_t, in0=var_t, in1=msq_t, op=Alu.subtract)
                # sd = sqrt(var + eps); rstd = 1/sd
                nc.scalar.activation(out=var_t, in_=var_t, func=Act.Sqrt,
                                     bias=eps_t, scale=1.0)
                rm_t = sb2.tile([1, 64], f32, tag="rm")
                nc.vector.reciprocal(out=rm_t[:, 0:32], in_=var_t)
                nc.vector.tensor_tensor(out=rm_t[:, 32:64], in0=mean_t,
                                        in1=rm_t[:, 0:32], op=Alu.mult)
                # broadcast across partitions: [128, 64]
                bc_ps = ps2.tile([128, 64], f32, tag="bc")
                nc.tensor.matmul(bc_ps, lhsT=ones1_t, rhs=rm_t, start=True, stop=True)
                nc.vector.tensor_copy(out=rmrb_t, in_=bc_ps)

                # G1Q/G1K[:, oc, t] = g1{q,k}[oc*128+p] * mr[t] - b{q,k}[oc*128+p]
                mr_ap = rm_t[:, 32:64]
                for g1t, bt, G1 in ((g1q_t, bq_t, G1Q_t), (g1k_t, bk_t, G1K_t)):
                    for oc in range(CC):
                        gq_ps = ps2.tile([128, T], f32, tag="gq", bufs=3)
                        nc.tensor.matmul(gq_ps, lhsT=g1t[:, 128 * oc:128 * (oc + 1)],
                                         rhs=mr_ap, start=True, stop=True)
                        nc.vector.tensor_scalar(
                            out=G1[:, oc, :], in0=gq_ps,
                            scalar1=bt[:, oc:oc + 1], scalar2=None,
                            op0=Alu.subtract)
                # CV[row, c] = mr[row % 32] * g1v[c] - bvb[row, c]
                mrpat_t = sb2.tile([1, 128], f32, tag="mrpat")
                mr_rep = bass.AP(tensor=rm_t.tensor, offset=rm_t.offset + 32,
                                 ap=[rm_t.ap[0], [0, 4], [1, 32]])
                nc.vector.tensor_copy(out=mrpat_t.rearrange("q (a b) -> q a b", a=4),
                                      in_=mr_rep)
                cv_ps = ps2.tile([128, C], f32, tag="cv")
                nc.tensor.matmul(cv_ps, lhsT=mrpat_t, rhs=g1v_t, start=True, stop=True)
                nc.vector.tensor_tensor(out=CV_t, in0=cv_ps, in1=bvb_t, op=Alu.subtract)

            # broadcast views: [128, 16(pix, step0), 32(t)]
            rbv = ap3(rmrb_t, 0, [[0, 16], [1, 32]])

            # ================= Phase 3: main =================
            with ExitStack() as p3:
                sb3 = p3.enter_context(tc.tile_pool(name="p3sb", bufs=1))
                mm_ps_pool = p3.enter_context(tc.tile_pool(name="mmps", bufs=6, space="PSUM"))

                po_ps_pool = p3.enter_context(tc.tile_pool(name="pops", bufs=2, space="PSUM"))

                for blk in range(NB):
                    # ---- load x block tiles [128, 32 t, 64 p]
                    x_t = []
                    for cc in range(CC):
                        xt = sb3.tile([128, T, PB], f32, tag="x", bufs=8)
                        nc.sync.dma_start(
                            out=xt,
                            in_=xv[cc * 128:(cc + 1) * 128, :, PB * blk:PB * (blk + 1)],
                        )
                        x_t.append(xt)

                    # per h-chunk state
                    for hc in range(4):
                        # ---- h = x*r - m*r   (pixel-major [128, 512] = 16 p x 32 t)
                        h_t = []
                        for cc in range(CC):
                            ht = sb3.tile([128, 512], f32r, tag="h", bufs=8)
                            h3 = ht.rearrange("q (p t) -> q p t", p=16)
                            xs = ap3(x_t[cc], 16 * hc, [[1, 16], [64, 32]])
                            nc.gpsimd.tensor_tensor(out=h3, in0=xs, in1=rbv, op=Alu.mult)
                            h_t.append(ht)

                        # ---- q, k GEMMs (output chunks [128, 512])
                        q_t, k_t = [], []
                        for wt, bt, dst in ((wq_t, bq_t, q_t), (wk_t, bk_t, k_t)):
                            for oc in range(CC):
                                mm_ps = mm_ps_pool.tile([128, 512], f32, tag="mm")
                                for cc in range(CC):
                                    nc.tensor.matmul(
                                        mm_ps,
                                        lhsT=wt[:, cc, 128 * oc:128 * (oc + 1)],
                                        rhs=h_t[cc],
                                        start=(cc == 0),
                                        stop=(cc == CC - 1),
                                    )
                                qt = sb3.tile([128, 512], f32r,
                                              tag=("q" if dst is q_t else "k"), bufs=8)
                                G1 = G1Q_t if dst is q_t else G1K_t
                                g1view = bass.AP(
                                    tensor=G1.tensor,
                                    offset=G1[:, oc, :].offset,
                                    ap=[G1.ap[0], [0, 16], [1, 32]])
                                nc.vector.scalar_tensor_tensor(
                                    out=qt.rearrange("q (p t) -> q p t", p=16),
                                    in0=mm_ps.rearrange("q (p t) -> q p t", p=16),
                                    scalar=1.0, in1=g1view,
                                    op0=Alu.mult, op1=Alu.subtract)
                                dst.append(qt)

                        # ---- vT per group (4 groups of 4 pixels in this h-chunk)
                        vt_t = []
                        for g in range(4):
                            mm_ps = mm_ps_pool.tile([128, 512], f32, tag="mm")
                            for cc in range(CC):
                                nc.tensor.matmul(
                                    mm_ps,
                                    lhsT=h_t[cc][:, 128 * g:128 * (g + 1)],
                                    rhs=wv_t[:, cc, :],
                                    start=(cc == 0),
                                    stop=(cc == CC - 1),
                                )
                            vt = sb3.tile([128, 512], bf16, tag="vt", bufs=6)
                            nc.vector.scalar_tensor_tensor(
                                out=vt, in0=mm_ps, scalar=1.0, in1=CV_t,
                                op0=Alu.mult, op1=Alu.subtract)
                            vt_t.append(vt)

                        # ---- attention, paired groups for 256-wide S rhs
                        out_big = sb3.tile([128, CC, 512], f32r, tag="out", bufs=2,
                                           name=f"out_{blk}_{hc}")
                        for gp in range(2):
                            gA, gB = 2 * gp, 2 * gp + 1
                            s_ps = {}
                            for g in (gA, gB):
                                sp = mm_ps_pool.tile([128, 256], f32, tag="mm")
                                for oc in range(CC):
                                    nc.tensor.matmul(
                                        sp,
                                        lhsT=q_t[oc][:, 128 * g:128 * (g + 1)],
                                        rhs=k_t[oc][:, 128 * gA:128 * gA + 256],
                                        start=(oc == 0),
                                        stop=(oc == CC - 1),
                                    )
                                s_ps[g] = sp
                            for g in (gA, gB):
                                half = g - gA
                                sm_t = sb3.tile([128, 128], f32, tag="sm", bufs=6)
                                nc.vector.tensor_tensor(
                                    out=sm_t, in0=s_ps[g][:, 128 * half:128 * (half + 1)],
                                    in1=mask_t, op=Alu.add)
                                p_t = sb3.tile([128, 128], bf16, tag="p", bufs=6)
                                rs_t = sb3.tile([128, 1], f32, tag="rs", bufs=4)
                                nc.scalar.activation(out=p_t, in_=sm_t, func=Act.Exp,
                                                     accum_out=rs_t)
                                ri_t = sb3.tile([128, 1], f32, tag="ri", bufs=4)
                                nc.vector.reciprocal(out=ri_t, in_=rs_t)
                                nc.vector.tensor_scalar_mul(out=p_t, in0=p_t, scalar1=ri_t)
                                pt_ps = po_ps_pool.tile([128, 128], bf16, tag="po")
                                nc.tensor.transpose(pt_ps, p_t, ident_t)
                                pt_t = sb3.tile([128, 128], bf16, tag="ptsb", bufs=6)
                                nc.scalar.copy(out=pt_t, in_=pt_ps)
                                out_ps = po_ps_pool.tile([128, CC, 128], f32, tag="po")
                                for cc in range(CC):
                                    nc.tensor.matmul(
                                        out_ps[:, cc, :],
                                        lhsT=vt_t[g][:, 128 * cc:128 * (cc + 1)],
                                        rhs=pt_t,
                                        start=True,
                                        stop=True,
                                    )
                                nc.vector.tensor_copy(
                                    out=out_big[:, :, 128 * g:128 * (g + 1)],
                                    in_=out_ps)

                        # ---- proj + residual into x tiles (in place)
                        for oc in range(CC):
                            mm_ps = mm_ps_pool.tile([128, 512], f32, tag="mm")
                            for cc in range(CC):
                                nc.tensor.matmul(
                                    mm_ps,
                                    lhsT=wp_t[:, cc, 128 * oc:128 * (oc + 1)],
                                    rhs=out_big[:, cc, :],
                                    start=(cc == 0),
                                    stop=(cc == CC - 1),
                                )
                            xres = ap3(x_t[oc], 16 * hc, [[1, 16], [64, 32]])
                            nc.vector.scalar_tensor_tensor(
                                out=xres,
                                in0=mm_ps.rearrange("q (p t) -> q p t", p=16),
                                scalar=1.0,
                                in1=xres,
                                op0=Alu.mult,
                                op1=Alu.add,
                            )

                    # ---- store block
                    for cc in range(CC):
                        nc.sync.dma_start(
                            out=yv[cc * 128:(cc + 1) * 128, :, PB * blk:PB * (blk + 1)],
                            in_=x_t[cc],
                        )

    nc.compile()
    return nc


def _host_prep(gamma, beta, wq, wk, wv, wproj):
    scale = float(C) ** -0.5
    g = gamma.astype(np.float64)
    b = beta.astype(np.float64)
    wq64 = wq.astype(np.float64)
    wk64 = wk.astype(np.float64)
    wv64 = wv.astype(np.float64)
    wqt = np.ascontiguousarray(((wq64 * g[None, :]) * scale).T.astype(np.float32))
    wkt = np.ascontiguousarray((wk64 * g[None, :]).T.astype(np.float32))
    wvt = np.ascontiguousarray((wv64 * g[None, :]).T.astype(np.float32))
    wpt = np.ascontiguousarray(wproj.astype(np.float32).T)
    bq = ((wq64 @ b) * scale).astype(np.float32)
    bk = (wk64 @ b).astype(np.float32)
    bv = (wv64 @ b).astype(np.float32)
    bvb = np.ascontiguousarray(np.broadcast_to(bv[None, :], (128, C)))
    g1q = np.ascontiguousarray(wqt.sum(axis=0, dtype=np.float64).astype(np.float32)[None, :])
    g1k = np.ascontiguousarray(wkt.sum(axis=0, dtype=np.float64).astype(np.float32)[None, :])
    g1v = np.ascontiguousarray(wvt.sum(axis=0, dtype=np.float64).astype(np.float32)[None, :])
    # additive causal/block-diag mask for [128 rows=(p,t), 128 cols=(p,s)]
    idx = np.arange(128)
    pi, ti = idx[:, None] // 32, idx[:, None] % 32
    pj, tj = idx[None, :] // 32, idx[None, :] % 32
    mask = np.where((pi == pj) & (tj <= ti), 0.0, -1e30).astype(np.float32)
    import ml_dtypes
    ident = np.eye(128, dtype=ml_dtypes.bfloat16)
    return wqt, wkt, wvt, wpt, bq, bk, bvb, mask, ident, g1q, g1k, g1v


def _get_runner():
    """Build (once) a sharded jitted executable for the compiled Bass module.

    Mirrors concourse.bass2jax.run_bass_via_pjrt's multi-core path, but keeps
    the jitted function so repeated calls don't retrace, and exposes enough
    structure for execution-only benchmarking.
    """
    if "runner" in _CACHE:
        return _CACHE["runner"]

    import jax
    from jax.sharding import Mesh, PartitionSpec
    from jax.experimental.shard_map import shard_map
    from concourse import bass2jax, mybir

    nc = _CACHE.get("nc")
    if nc is None:
        nc = _build()
        _CACHE["nc"] = nc

    bass2jax.install_neuronx_cc_hook()

    partition_name = nc.partition_id_tensor.name if nc.partition_id_tensor else None
    in_names, out_names, out_avals = [], [], []
    for alloc in nc.m.functions[0].allocations:
        if not isinstance(alloc, mybir.MemoryLocationSet):
            continue
        name = alloc.memorylocations[0].name
        if alloc.kind == "ExternalInput":
            if name != partition_name:
                in_names.append(name)
        elif alloc.kind == "ExternalOutput":
            out_names.append(name)
            shape = tuple(alloc.tensor_shape)
            dtype = mybir.dt.np(alloc.dtype)
            out_avals.append(jax.core.ShapedArray(shape, dtype))
    n_params = len(in_names)
    n_outs = len(out_avals)
    all_in_names = list(in_names) + list(out_names)
    if partition_name is not None:
        all_in_names.append(partition_name)
    donate = tuple(range(n_params, n_params + n_outs))

    def _body(*args):
        operands = list(args)
        if partition_name is not None:
            operands.append(bass2jax.partition_id_tensor())
        outs = bass2jax._bass_exec_p.bind(
            *operands,
            out_avals=tuple(out_avals),
            in_names=tuple(all_in_names),
            out_names=tuple(out_names),
            lowering_input_output_aliases=(),
            sim_require_finite=True,
            sim_require_nnan=True,
            nc=nc,
        )
        return tuple(outs)

    devices = jax.devices()[:8]
    mesh = Mesh(np.asarray(devices), ("core",))
    in_specs = (PartitionSpec("core"),) * (n_params + n_outs)
    out_specs = (PartitionSpec("core"),) * n_outs
    sharded = jax.jit(
        shard_map(_body, mesh=mesh, in_specs=in_specs, out_specs=out_specs,
                  check_rep=False),
        donate_argnums=donate,
        keep_unused=True,
    )
    runner = {
        "sharded": sharded,
        "mesh": mesh,
        "in_names": in_names,
        "out_names": out_names,
        "out_avals": out_avals,
    }
    _CACHE["runner"] = runner
    return runner


def _run(in_maps):
    runner = _get_runner()
    in_names = runner["in_names"]
    out_avals = runner["out_avals"]
    per_core = [[np.asarray(m[name]) for name in in_names] for m in in_maps]
    concat_in = [
        np.concatenate([per_core[c][i] for c in range(8)], axis=0)
        for i in range(len(in_names))
    ]
    concat_zeros = [
        np.zeros((8 * a.shape[0], *a.shape[1:]), a.dtype) for a in out_avals
    ]
    out_arrs = runner["sharded"](*concat_in, *concat_zeros)
    _CACHE["last_run"] = (concat_in, [tuple(a.shape) for a in out_avals])
    return [
        {
            name: np.asarray(out_arrs[i]).reshape(8, *out_avals[i].shape)[c]
            for i, name in enumerate(runner["out_names"])
        }
        for c in range(8)
    ]


def bench(iters=10):
    """Time device-side execution with inputs pre-staged on the devices."""
    import time
    import jax
    from jax.sharding import NamedSharding, PartitionSpec

    runner = _get_runner()
    concat_in, out_shapes = _CACHE["last_run"]
    sharding = NamedSharding(runner["mesh"], PartitionSpec("core"))
    dev_in = [jax.device_put(a, sharding) for a in concat_in]
    for a in dev_in:
        a.block_until_ready()
    times = []
    for _ in range(iters):
        zeros = [
            jax.device_put(np.zeros((8 * s[0], *s[1:]), np.float32), sharding)
            for s in out_shapes
        ]
        for z in zeros:
            z.block_until_ready()
        t0 = time.perf_counter()
        outs = runner["sharded"](*dev_in, *zeros)
        for o in outs:
            o.block_until_ready()
        t1 = time.perf_counter()
        times.append(t1 - t0)
    return times


def kernel(x, gamma, beta, wq, wk, wv, wproj):
    x = np.asarray(x, dtype=np.float32)
    gamma = np.asarray(gamma, dtype=np.float32)
    beta = np.asarray(beta, dtype=np.float32)
    wq = np.asarray(wq, dtype=np.float32)
    wk = np.asarray(wk, dtype=np.float32)
    wv = np.asarray(wv, dtype=np.float32)
    wproj = np.asarray(wproj, dtype=np.float32)

    wqt, wkt, wvt, wpt, bq, bk, bvb, mask, ident, g1q, g1k, g1v = _host_prep(
        gamma, beta, wq, wk, wv, wproj)

    B = x.shape[0]
    in_maps = []
    for i in range(8):
        b, h0 = i // 2, HL * (i % 2)
        in_maps.append({
            "x": np.ascontiguousarray(x[b, :, :, h0:h0 + HL, :]),
            "wqt": wqt, "wkt": wkt, "wvt": wvt, "wpt": wpt,
            "bq": bq, "bk": bk, "bvb": bvb, "mask": mask, "ident": ident,
            "ones": np.ones((128, 32), dtype=np.float32),
            "g1q": g1q, "g1k": g1k, "g1v": g1v,
        })

    results = _run(in_maps)

    y = np.empty((B, C, T, 2 * HL, W), dtype=np.float32)
    for i in range(8):
        b, h0 = i // 2, HL * (i % 2)
        y[b, :, :, h0:h0 + HL, :] = results[i]["y"]
    return y
